# revision 1
# baseline (speedup 1.0000x reference)
"""ExtendedANFIS forward, p=9 factored rules, per-sub pipelined combine.

    r = r1*243 + r2   (P over inputs {0,1} -> 9 rules, Q over {2..6} -> 243)
    out[b,o] = (sum_r1 P[b,r1] * C[b,(o,r1)]) / (ones-col + eps)
    C[b,(o,r1)] = sum_r2 Q[b,r2] * R2[r2, o*9+r1]

Layout: Lpos lives on 15 contiguous partitions (k = 5m + i-2), so the
Sel matmul contracts K=15 and there are no pad rows to zero; x^T comes
from one PE transpose per (group, nt) straight into the Square input.
PSUM per group: ca [128,4,512] (o 0..55, 4 banks) + cbz [128,4,81]
(o 56..64 incl ones-col, 1 bank) + pq/px shared tag (2 banks).
"""

import numpy as np

NC = 8
B = 16384
BC = B // NC          # 2048 rows per core
I = 7
M = 3
O = 64
R1 = 9
R2N = 243
KB = 21               # Lpos rows: k = 7m + i (all inputs; P uses i<2)
W = (O + 1) * R1      # 585 real columns, col = o*9 + r1
WA = 56 * R1          # 504 (o 0..55) - f32r part, N even >=256
WB = W - WA           # 81 (o 56..64) - bf16 part
NG = 4
GB = 512
EPS = 1e-6

GPS_CB = False

_CACHE = {}


def _build_sel5() -> np.ndarray:
    """[21, 256] f32: row 7m+i (i in 2..6), col r2 -> -1 where digit==m."""
    dg = np.stack(np.unravel_index(np.arange(R2N), (M,) * 5), axis=0)
    sel = np.zeros((KB, 256), dtype=np.float32)
    for m in range(M):
        for j in range(5):
            sel[7 * m + (j + 2), :R2N] = -(dg[j] == m).astype(np.float32)
    return sel


def _build_selp() -> np.ndarray:
    """[21, 10] f32: logP selector; col r1=3*m0+m1 <- -1 at rows (m0,i=0),(m1,i=1)."""
    sel = np.zeros((KB, 10), dtype=np.float32)
    for m0 in range(M):
        for m1 in range(M):
            sel[7 * m0 + 0, 3 * m0 + m1] = -1.0
            sel[7 * m1 + 1, 3 * m0 + m1] = -1.0
    return sel


def _build_program(repeat: int = 1, unroll: bool = False):
    import concourse.mybir as mybir
    import concourse.tile as tile
    from concourse import bacc
    from concourse.bass import DRamTensorHandle

    fp32 = mybir.dt.float32
    bf16 = mybir.dt.bfloat16
    f32r = mybir.dt.float32r
    AF = mybir.ActivationFunctionType
    ALU = mybir.AluOpType

    nc = bacc.Bacc(None, target_bir_lowering=False, debug=False)

    xs = nc.dram_tensor("xs", [BC, I], fp32, kind="ExternalInput")
    mfs_d = nc.dram_tensor("mfs", [I, M, 2], fp32, kind="ExternalInput")
    r2_d = nc.dram_tensor("r2mat", [256, W], fp32, kind="ExternalInput")
    out_d = nc.dram_tensor("out", [128, BC // 128, O], fp32, kind="ExternalOutput")

    sel_np = _build_sel5()
    sel_c = nc.inline_tensor(sel_np, name="sel5")
    nc.lookup_mls(sel_c).dtype = f32r
    sel_c = DRamTensorHandle("sel5", list(sel_np.shape), f32r)
    selp_np = _build_selp()
    selp_c = nc.inline_tensor(selp_np, name="selp")
    nc.lookup_mls(selp_c).dtype = f32r
    selp_c = DRamTensorHandle("selp", list(selp_np.shape), f32r)
    id_c = nc.inline_tensor(np.eye(128, dtype=np.float32), name="ident")

    with tile.TileContext(nc) as tc:
        with (
            tc.tile_pool(name="consts", bufs=1) as consts,
            tc.tile_pool(name="work", bufs=2) as work,
            tc.tile_pool(name="psQ", bufs=2, space="PSUM") as psQ,
            tc.tile_pool(name="psA", bufs=1, space="PSUM") as psA,
            tc.tile_pool(name="psB", bufs=2, space="PSUM") as psB,
        ):
            # ---- constants / input staging ----
            # xallQ[p, g, nt, (m,i)] = xs[(4g+nt)*128+p, 2+i], x replicated
            # 3x (once per m) so one transpose yields all 15 Lpos rows.
            xallQ = consts.tile([128, NG, 4, KB], fp32)
            for g in range(NG):
                for m in range(M):
                    nc.sync.dma_start(
                        out=xallQ[:, g, :, 7 * m : 7 * m + 7],
                        in_=xs[g * GB : (g + 1) * GB, :].rearrange(
                            "(nt p) i -> p nt i", nt=4, p=128
                        ),
                    )
            sel5 = consts.tile([KB, 256], f32r)
            nc.scalar.dma_start(out=sel5[:, 0:128], in_=sel_c[:, 0:128])
            nc.scalar.dma_start(out=sel5[:, 128:], in_=sel_c[:, 128:])
            selp = consts.tile([KB, 10], f32r)
            nc.scalar.dma_start(out=selp, in_=selp_c[:, :])
            ident = consts.tile([128, 128], fp32)
            nc.sync.dma_start(out=ident, in_=id_c[:, :])
            r2f = consts.tile([128, 2, W], fp32)
            for c in range(2):
                nc.gpsimd.dma_start(
                    out=r2f[:, c], in_=r2_d[128 * c : 128 * (c + 1), :]
                )
            r2a = consts.tile([128, 2, WA], bf16)
            nc.vector.tensor_copy(out=r2a, in_=r2f[:, :, 0:WA])
            r2b = consts.tile([128, 2, WB + 1], bf16)
            nc.vector.memset(r2b, 0.0)
            nc.vector.tensor_copy(out=r2b[:, :, 0:WB], in_=r2f[:, :, WA:W])

            # mfs -> c/d per Lpos row (k = 7m + i)
            mtile = consts.tile([KB, 2], fp32)
            for m in range(M):
                nc.scalar.dma_start(
                    out=mtile[7 * m : 7 * m + 7, :], in_=mfs_d[:, m, :]
                )
            tmp = consts.tile([KB, 1], fp32)
            cvec = consts.tile([KB, 1], fp32)
            dvec = consts.tile([KB, 1], fp32)
            nc.vector.tensor_scalar_mul(tmp, mtile[:, 1:2], -1.0)
            nc.vector.tensor_tensor(tmp, mtile[:, 1:2], tmp, ALU.max)
            nc.vector.tensor_scalar_add(tmp, tmp, EPS)
            nc.vector.reciprocal(cvec, tmp)
            nc.vector.tensor_scalar_mul(cvec, cvec, float(1.0 / np.sqrt(2.0)))
            nc.vector.tensor_tensor(dvec, mtile[:, 0:1], cvec, ALU.mult)
            nc.vector.tensor_scalar_mul(dvec, dvec, -1.0)


            def body():
                lpos = work.tile([KB, BC], f32r, tag="lpos", bufs=2)
                for g in range(NG):
                    gs = slice(g * GB, (g + 1) * GB)
                    # -- x^T bands + Lpos
                    px = psQ.tile([KB, 4, 128], fp32, tag="pq", bufs=2)
                    for nt in range(4):
                        nc.tensor.transpose(px[:, nt], xallQ[:, g, nt, :], ident)
                    nc.scalar.activation(
                        out=lpos[:, gs],
                        in_=px[:, :].rearrange("p nt c -> p (nt c)"),
                        func=AF.Square, scale=cvec, bias=dvec,
                    )
                    # -- logQ^T (2 chunks of 128) and exp -> Q^T
                    qt = work.tile([128, 2, GB], bf16, tag="qt", bufs=3)
                    for c in range(2):
                        pq = psQ.tile([128, GB], fp32, tag="pq", bufs=2)
                        nc.tensor.matmul(
                            pq, sel5[:, 128 * c : 128 * (c + 1)], lpos[:, gs],
                            start=True, stop=True,
                        )
                        nc.scalar.activation(out=qt[:, c], in_=pq, func=AF.Exp)
                    # -- P path: logP = SelP^T @ Lpos per sub, then exp
                    psp = psB.tile([128, 4, 10], fp32, tag="psp", bufs=1)
                    for s in range(4):
                        nc.tensor.matmul(
                            psp[:, s],
                            lpos[:, g * GB + s * 128 : g * GB + (s + 1) * 128],
                            selp, start=True, stop=True,
                            skip_group_check=True,
                        )
                    ptile = work.tile([128, 4, 10], fp32, tag="ptile", bufs=3)
                    nc.scalar.activation(out=ptile, in_=psp, func=AF.Exp)
                    ptb = work.tile([128, 4, 10], bf16, tag="ptb", bufs=3)
                    nc.scalar.activation(out=ptb, in_=psp, func=AF.Exp)

                    # -- C matmuls + per-sub combine mults (pipelined)
                    dtile = work.tile([128, 4, W], bf16, tag="d", bufs=3)
                    cb = psB.tile([128, 4, WB], fp32, tag="cb", bufs=1)
                    for s in range(4):
                        ca = psA.tile([128, 512], fp32, tag="ca", bufs=3)
                        for c in range(2):
                            lhsT = qt[:, c, s * 128 : (s + 1) * 128]
                            nc.tensor.matmul(
                                ca[:, 0:WA], lhsT, r2a[:, c],
                                start=(c == 0), stop=(c == 1),
                                skip_group_check=True,
                            )
                            nc.tensor.matmul(
                                cb[:, s], lhsT, r2b[:, c, 0:WB],
                                start=(c == 0), stop=(c == 1),
                                skip_group_check=True,
                            )
                        pba = (
                            ptile[:, s, 0:R1].unsqueeze(1)
                            .broadcast_to([128, 56, R1])
                        )
                        if s < 2:
                            # ACT evacuates psum->bf16; all-bf16 operands
                            # (incl. the bf16 P twin) let DVE run at 2x
                            cev = work.tile([128, WA], bf16, tag="cev", bufs=2)
                            nc.scalar.copy(out=cev, in_=ca[:, 0:WA])
                            pbab = (
                                ptb[:, s, 0:R1].unsqueeze(1)
                                .broadcast_to([128, 56, R1])
                            )
                            nc.vector.tensor_tensor(
                                dtile[:, s, 0:WA].rearrange(
                                    "p (o r) -> p o r", r=R1
                                ),
                                cev[:, :].rearrange("p (o r) -> p o r", r=R1),
                                pbab, ALU.mult,
                            )
                        else:
                            nc.vector.tensor_tensor(
                                dtile[:, s, 0:WA].rearrange(
                                    "p (o r) -> p o r", r=R1
                                ),
                                ca[:, 0:WA].rearrange("p (o r) -> p o r", r=R1),
                                pba, ALU.mult,
                            )


                    # cb combine for the whole group in one op
                    cbeng = nc.gpsimd if GPS_CB else nc.vector
                    cbeng.tensor_tensor(
                        dtile[:, :, WA:W].rearrange("p s (o r) -> p s o r", r=R1),
                        cb[:, :, :].rearrange("p s (o r) -> p s o r", r=R1),
                        ptile[:, :, 0:R1].unsqueeze(2).broadcast_to(
                            [128, 4, 9, R1]
                        ),
                        ALU.mult,
                    )
                    # -- 9->1 tree + normalize + store
                    dv = dtile[:, :, :].rearrange("p s (o r) -> p s o r", r=R1)
                    t1 = work.tile([128, 4, O + 1, 4], bf16, tag="t1", bufs=3)
                    t2 = work.tile([128, 4, O + 1, 2], bf16, tag="t2", bufs=3)
                    et = work.tile([128, 4, O + 1], fp32, tag="et", bufs=3)
                    nc.vector.tensor_tensor(
                        t1, dv[:, :, :, 0:4], dv[:, :, :, 4:8], ALU.add
                    )
                    nc.vector.tensor_tensor(
                        t2, t1[:, :, :, 0:2], t1[:, :, :, 2:4], ALU.add
                    )
                    nc.vector.tensor_tensor(
                        et, t2[:, :, :, 0], dv[:, :, :, 8], ALU.add
                    )
                    nc.vector.tensor_tensor(et, et, t2[:, :, :, 1], ALU.add)
                    zt = work.tile([128, 4, 1], fp32, tag="zt", bufs=3)
                    nc.vector.tensor_scalar_add(zt, et[:, :, 64:65], EPS)
                    nc.vector.reciprocal(zt, zt)
                    ot = work.tile([128, 4, O], fp32, tag="ot", bufs=3)
                    nc.vector.tensor_tensor(
                        ot, et[:, :, 0:O], zt.broadcast_to([128, 4, O]), ALU.mult
                    )
                    eng = (nc.sync, nc.scalar)[g % 2]
                    eng.dma_start(
                        out=out_d[:, 4 * g : 4 * (g + 1), :], in_=ot
                    )

            if repeat == 1:
                body()
            elif unroll:
                for _ in range(repeat):
                    body()
            else:
                inner = 16 if repeat % 16 == 0 else (4 if repeat % 4 == 0 else 1)
                with tc.For_i(0, repeat // inner, 1, staggered_reset=True):
                    for _ in range(inner):
                        body()

    nc.finalize()
    return nc


def _host_prep(x, mfs, rules):
    x = np.ascontiguousarray(x, dtype=np.float32)
    mfs = np.ascontiguousarray(mfs, dtype=np.float32)
    ra = np.zeros((R1 * R2N, O + 1), dtype=np.float32)
    ra[:, :O] = rules
    ra[:, O] = 1.0
    r2mat = np.zeros((256, W), dtype=np.float32)
    r2mat[:R2N] = ra.reshape(R1, R2N, O + 1).transpose(1, 2, 0).reshape(R2N, W)
    return x, mfs, r2mat


def _make_runner(nc):
    """Jitted 8-core SPMD runner (compiles once, reused across calls)."""
    import jax
    from jax.sharding import Mesh, PartitionSpec
    from jax.experimental.shard_map import shard_map

    import concourse.mybir as mybir
    from concourse import bass2jax
    from concourse.bass2jax import _bass_exec_p, install_neuronx_cc_hook

    install_neuronx_cc_hook()
    partition_name = nc.partition_id_tensor.name if nc.partition_id_tensor else None
    in_names, out_names, out_avals, out_shapes = [], [], [], []
    for alloc in nc.m.functions[0].allocations:
        if not isinstance(alloc, mybir.MemoryLocationSet):
            continue
        name = alloc.memorylocations[0].name
        if alloc.kind == "ExternalInput":
            if name != partition_name:
                in_names.append(name)
        elif alloc.kind == "ExternalOutput":
            out_names.append(name)
            shape = tuple(alloc.tensor_shape)
            dtype = mybir.dt.np(alloc.dtype)
            out_avals.append(jax.core.ShapedArray(shape, dtype))
            out_shapes.append((shape, dtype))
    n_params = len(in_names)
    n_outs = len(out_avals)
    all_in = list(in_names) + list(out_names)
    if partition_name is not None:
        all_in.append(partition_name)
    donate = tuple(range(n_params, n_params + n_outs))

    def _fn(*args):
        operands = list(args)
        if partition_name is not None:
            operands.append(bass2jax.partition_id_tensor())
        return tuple(
            _bass_exec_p.bind(
                *operands,
                out_avals=tuple(out_avals),
                in_names=tuple(all_in),
                out_names=tuple(out_names),
                lowering_input_output_aliases=(),
                sim_require_finite=True,
                sim_require_nnan=True,
                nc=nc,
            )
        )

    devices = jax.devices()[:NC]
    mesh = Mesh(np.asarray(devices), ("core",))
    spec = (PartitionSpec("core"),)
    sharded = jax.jit(
        shard_map(
            _fn, mesh=mesh, in_specs=spec * (n_params + n_outs),
            out_specs=spec * n_outs, check_rep=False,
        ),
        donate_argnums=donate, keep_unused=True,
    )

    def run(in_maps, fetch=True):
        import jax

        concat_in = [
            np.concatenate([np.asarray(m[n]) for m in in_maps], axis=0)
            for n in in_names
        ]
        zeros = [np.zeros((NC * s[0], *s[1:]), dt) for s, dt in out_shapes]
        out_arrs = sharded(*concat_in, *zeros)
        if not fetch:
            jax.block_until_ready(out_arrs)
            return None
        return {
            n: np.asarray(out_arrs[i]).reshape(NC, *out_avals[i].shape)
            for i, n in enumerate(out_names)
        }

    return run


def kernel(x: np.ndarray, mfs: np.ndarray, rules: np.ndarray) -> np.ndarray:
    make_runner = _make_runner
    key = "runner"
    if key not in _CACHE:
        _CACHE[key] = make_runner(_build_program())
    run = _CACHE[key]

    x, mfs, r2mat = _host_prep(x, mfs, rules)
    in_maps = [
        {"xs": x[c * BC : (c + 1) * BC], "mfs": mfs, "r2mat": r2mat}
        for c in range(NC)
    ]
    outs = run(in_maps)
    o = outs["out"]  # [NC, 128, BC//128, O]
    o = o.transpose(0, 2, 1, 3).reshape(B, O)
    return np.ascontiguousarray(o)



# revision 3
# speedup vs baseline: 1.0006x; 1.0006x over previous
"""ExtendedANFIS forward, p=9 factored rules, engine-balanced combine.

    r = r1*243 + r2   (P over inputs {0,1} -> 9 rules, Q over {2..6} -> 243)
    out[b,o] = (sum_r1 P[b,r1] * C[b,(o,r1)]) / (ones-col + eps)
    C[b,(o,r1)] = sum_r2 Q[b,r2] * R2[r2, o*9+r1]

v2 layout/schedule:
- x^T transposes + Lpos Square hoisted to setup (loop-invariant).
- P-matmul PSUM rides in the pq tile's 2nd bank; one Exp covers Q chunk0+P.
- Per-sub combine split: subs 0,2 ACT-evac -> all-bf16 DVE mult (2x);
  subs 1,3 Pool (gpsimd) mult direct from PSUM.
- 9->1 reduction tree all-bf16 on DVE (2x); cb combine on DVE.
"""

import numpy as np

NC = 8
B = 16384
BC = B // NC          # 2048 rows per core
I = 7
M = 3
O = 64
R1 = 9
R2N = 243
KB = 21               # Lpos rows: k = 7m + i (all inputs; P uses i<2)
W = (O + 1) * R1      # 585 real columns, col = o*9 + r1
WA = 56 * R1          # 504 (o 0..55)
WB = W - WA           # 81 (o 56..64 incl ones-col)
NG = 4
GB = 512
EPS = 1e-6

_CACHE = {}


def _build_sel5() -> np.ndarray:
    """[21, 256] f32: row 7m+i (i in 2..6), col r2 -> -1 where digit==m."""
    dg = np.stack(np.unravel_index(np.arange(R2N), (M,) * 5), axis=0)
    sel = np.zeros((KB, 256), dtype=np.float32)
    for m in range(M):
        for j in range(5):
            sel[7 * m + (j + 2), :R2N] = -(dg[j] == m).astype(np.float32)
    return sel


def _build_selp() -> np.ndarray:
    """[21, 10] f32: logP selector; col r1=3*m0+m1 <- -1 at rows (m0,i=0),(m1,i=1)."""
    sel = np.zeros((KB, 10), dtype=np.float32)
    for m0 in range(M):
        for m1 in range(M):
            sel[7 * m0 + 0, 3 * m0 + m1] = -1.0
            sel[7 * m1 + 1, 3 * m0 + m1] = -1.0
    return sel


def _build_program(repeat: int = 1, unroll: bool = False):
    import concourse.mybir as mybir
    import concourse.tile as tile
    from concourse import bacc
    from concourse.bass import DRamTensorHandle

    fp32 = mybir.dt.float32
    bf16 = mybir.dt.bfloat16
    f32r = mybir.dt.float32r
    AF = mybir.ActivationFunctionType
    ALU = mybir.AluOpType

    import os as _os
    _CA_BUFS = int(_os.environ.get("CA_BUFS", "4"))
    _PQ_BUFS = int(_os.environ.get("PQ_BUFS", "2"))
    _T1_SPLIT = _os.environ.get("T1_SPLIT", "0") == "1"

    nc = bacc.Bacc(None, target_bir_lowering=False, debug=False)

    xs = nc.dram_tensor("xs", [BC, I], fp32, kind="ExternalInput")
    mfs_d = nc.dram_tensor("mfs", [I, M, 2], fp32, kind="ExternalInput")
    r2_d = nc.dram_tensor("r2mat", [256, W], fp32, kind="ExternalInput")
    out_d = nc.dram_tensor("out", [128, BC // 128, O], fp32, kind="ExternalOutput")

    sel_np = _build_sel5()
    sel_c = nc.inline_tensor(sel_np, name="sel5")
    nc.lookup_mls(sel_c).dtype = f32r
    sel_c = DRamTensorHandle("sel5", list(sel_np.shape), f32r)
    selp_np = _build_selp()
    selp_c = nc.inline_tensor(selp_np, name="selp")
    nc.lookup_mls(selp_c).dtype = f32r
    selp_c = DRamTensorHandle("selp", list(selp_np.shape), f32r)
    id_c = nc.inline_tensor(np.eye(128, dtype=np.float32), name="ident")

    with tile.TileContext(nc) as tc:
        with (
            tc.tile_pool(name="consts", bufs=1) as consts,
            tc.tile_pool(name="work", bufs=2) as work,
            tc.tile_pool(name="psQ", bufs=2, space="PSUM") as psQ,
            tc.tile_pool(name="psA", bufs=1, space="PSUM") as psA,
            tc.tile_pool(name="psB", bufs=1, space="PSUM") as psB,
        ):
            # ---- constants / input staging (all loop-invariant) ----
            # xallQ[p, g, nt, (m,i)] = xs[(4g+nt)*128+p, 2+i], x replicated
            # 3x (once per m) so one transpose yields all 21 Lpos rows.
            xallQ = consts.tile([128, NG, 4, KB], fp32)
            for g in range(NG):
                for m in range(M):
                    nc.sync.dma_start(
                        out=xallQ[:, g, :, 7 * m : 7 * m + 7],
                        in_=xs[g * GB : (g + 1) * GB, :].rearrange(
                            "(nt p) i -> p nt i", nt=4, p=128
                        ),
                    )
            sel5 = consts.tile([KB, 256], f32r)
            nc.scalar.dma_start(out=sel5[:, 0:128], in_=sel_c[:, 0:128])
            nc.scalar.dma_start(out=sel5[:, 128:], in_=sel_c[:, 128:])
            selp = consts.tile([KB, 10], f32r)
            nc.scalar.dma_start(out=selp, in_=selp_c[:, :])
            ident = consts.tile([128, 128], fp32)
            nc.sync.dma_start(out=ident, in_=id_c[:, :])
            r2f = consts.tile([128, 2, W], fp32)
            for c in range(2):
                nc.gpsimd.dma_start(
                    out=r2f[:, c], in_=r2_d[128 * c : 128 * (c + 1), :]
                )
            r2a = consts.tile([128, 2, WA], bf16)
            nc.vector.tensor_copy(out=r2a, in_=r2f[:, :, 0:WA])
            r2b = consts.tile([128, 2, WB + 1], bf16)
            nc.vector.memset(r2b, 0.0)
            nc.vector.tensor_copy(out=r2b[:, :, 0:WB], in_=r2f[:, :, WA:W])

            # mfs -> c/d per Lpos row (k = 7m + i)
            mtile = consts.tile([KB, 2], fp32)
            for m in range(M):
                nc.scalar.dma_start(
                    out=mtile[7 * m : 7 * m + 7, :], in_=mfs_d[:, m, :]
                )
            tmp = consts.tile([KB, 1], fp32)
            cvec = consts.tile([KB, 1], fp32)
            dvec = consts.tile([KB, 1], fp32)
            nc.vector.tensor_scalar_mul(tmp, mtile[:, 1:2], -1.0)
            nc.vector.tensor_tensor(tmp, mtile[:, 1:2], tmp, ALU.max)
            nc.vector.tensor_scalar_add(tmp, tmp, EPS)
            nc.vector.reciprocal(cvec, tmp)
            nc.vector.tensor_scalar_mul(cvec, cvec, float(1.0 / np.sqrt(2.0)))
            nc.vector.tensor_tensor(dvec, mtile[:, 0:1], cvec, ALU.mult)
            nc.vector.tensor_scalar_mul(dvec, dvec, -1.0)

            # Lpos [21, BC] computed once: x^T via PE transpose, then
            # Square(scale*x + bias) per Lpos row on ACT. px borrows a
            # pq-tagged PSUM buffer (setup only) to save a bank.
            lpos = consts.tile([KB, BC], f32r)
            for g in range(NG):
                pxt = psQ.tile([128, 512], fp32, tag="pq", bufs=_PQ_BUFS)
                px = pxt[0:KB, 0:512].rearrange("p (nt c) -> p nt c", nt=4)
                for nt in range(4):
                    nc.tensor.transpose(px[:, nt], xallQ[:, g, nt, :], ident)
                nc.scalar.activation(
                    out=lpos[:, g * GB : (g + 1) * GB],
                    in_=px[:, :].rearrange("p nt c -> p (nt c)"),
                    func=AF.Square, scale=cvec, bias=dvec,
                )

            def stage1(g):
                """Selector matmuls + exps: logQ chunk0 [128,512] + logP
                [128,4,10] share the pq tile (banks 0/1); one Exp covers
                both. Returns (qp, qt2, ptb)."""
                gs = slice(g * GB, (g + 1) * GB)
                # cbp bank: cb rules-part [0:324] + logP psum [324:364]
                cbp = psB.tile([128, 364], fp32, tag="cbp", bufs=2)
                pq = psQ.tile([128, 512], fp32, tag="pq", bufs=_PQ_BUFS)
                nc.tensor.matmul(
                    pq, sel5[:, 0:128], lpos[:, gs],
                    start=True, stop=True,
                )
                for s in range(4):
                    nc.tensor.matmul(
                        cbp[:, 324 + s * 10 : 324 + s * 10 + 10],
                        lpos[:, g * GB + s * 128 : g * GB + (s + 1) * 128],
                        selp, start=True, stop=True,
                        skip_group_check=True,
                    )
                qp = work.tile([128, 512], bf16, tag="qp", bufs=3)
                nc.scalar.activation(out=qp, in_=pq, func=AF.Exp)
                ptb = work.tile([128, 4, 10], bf16, tag="ptb", bufs=3)
                nc.scalar.activation(
                    out=ptb, in_=cbp[:, 324:364].rearrange(
                        "p (s r) -> p s r", s=4), func=AF.Exp)
                pq2 = psQ.tile([128, 512], fp32, tag="pq", bufs=_PQ_BUFS)
                nc.tensor.matmul(
                    pq2, sel5[:, 128:256], lpos[:, gs],
                    start=True, stop=True,
                )
                qt2 = work.tile([128, 512], bf16, tag="qt2", bufs=3)
                nc.scalar.activation(out=qt2, in_=pq2, func=AF.Exp)
                return qp, qt2, ptb, cbp

            def stage2(g, st):
                """C matmuls + per-sub combine: s0,s2 Pool direct-from-PSUM;
                s1,s3 ACT evac + DVE bf16 mult (2x); cb Pool-evac + DVE
                mult. Returns (dtile, ptb)."""
                qp, qt2, ptb, cbp = st
                dtile = work.tile([128, 4, W], bf16, tag="d", bufs=3)
                cb = cbp[:, 0:324].rearrange("p (s w) -> p s w", s=4)
                for s in range(4):
                    ca = psA.tile([128, 512], fp32, tag="ca", bufs=_CA_BUFS)
                    for c in range(2):
                        lhsT = (qp, qt2)[c][:, s * 128 : (s + 1) * 128]
                        nc.tensor.matmul(
                            ca[:, 0:WA], lhsT, r2a[:, c],
                            start=(c == 0), stop=(c == 1),
                            skip_group_check=True,
                        )
                        nc.tensor.matmul(
                            cb[:, s], lhsT, r2b[:, c, 0:WB],
                            start=(c == 0), stop=(c == 1),
                            skip_group_check=True,
                        )
                    pba = (
                        ptb[:, s, 0:R1].unsqueeze(1)
                        .broadcast_to([128, 56, R1])
                    )
                    if s in (0, 2):
                        # DVE mult direct from PSUM (Pool cannot touch PSUM)
                        nc.vector.tensor_tensor(
                            dtile[:, s, 0:WA].rearrange(
                                "p (o r) -> p o r", r=R1
                            ),
                            ca[:, 0:WA].rearrange("p (o r) -> p o r", r=R1),
                            pba, ALU.mult,
                        )
                    else:
                        # ACT evacuates psum->bf16; Pool bf16 mult (SBUF-only)
                        cev = work.tile([128, WA], bf16, tag="cev", bufs=3)
                        nc.scalar.copy(out=cev, in_=ca[:, 0:WA])
                        nc.gpsimd.tensor_tensor(
                            dtile[:, s, 0:WA].rearrange(
                                "p (o r) -> p o r", r=R1
                            ),
                            cev[:, :].rearrange("p (o r) -> p o r", r=R1),
                            pba, ALU.mult,
                        )
                if _T1_SPLIT:
                    t1 = work.tile([128, 4, O + 1, 4], bf16, tag="t1", bufs=2)
                    for s in range(4):
                        dvs = dtile[:, s, :].rearrange(
                            "p (o r) -> p o r", r=R1)
                        nc.vector.tensor_tensor(
                            t1[:, s], dvs[:, :, 0:4], dvs[:, :, 4:8], ALU.add
                        )
                else:
                    t1 = None
                # cb: DVE mult direct from PSUM
                nc.vector.tensor_tensor(
                    dtile[:, :, WA:W].rearrange("p s (o r) -> p s o r", r=R1),
                    cb.rearrange("p s (o r) -> p s o r", r=R1),
                    ptb[:, :, 0:R1].unsqueeze(2).broadcast_to(
                        [128, 4, 9, R1]
                    ),
                    ALU.mult,
                )
                return dtile, t1

            def stage3(g, st2):
                """9->1 tree all-bf16 (DVE 2x) + normalize + store."""
                dtile, t1 = st2
                dv = dtile[:, :, :].rearrange("p s (o r) -> p s o r", r=R1)
                t2 = work.tile([128, 4, O + 1, 2], bf16, tag="t2", bufs=2)
                e1 = work.tile([128, 4, O + 1], bf16, tag="e1", bufs=2)
                et = work.tile([128, 4, O + 1], bf16, tag="et", bufs=2)
                if t1 is None:
                    t1 = work.tile([128, 4, O + 1, 4], bf16, tag="t1", bufs=2)
                    nc.vector.tensor_tensor(
                        t1, dv[:, :, :, 0:4], dv[:, :, :, 4:8], ALU.add
                    )
                nc.vector.tensor_tensor(
                    t2, t1[:, :, :, 0:2], t1[:, :, :, 2:4], ALU.add
                )
                nc.vector.tensor_tensor(
                    e1, t2[:, :, :, 0], t2[:, :, :, 1], ALU.add
                )
                nc.vector.tensor_tensor(et, e1, dv[:, :, :, 8], ALU.add)
                zt = work.tile([128, 4, 1], fp32, tag="zt", bufs=2)
                nc.vector.tensor_scalar_add(zt, et[:, :, 64:65], EPS)
                nc.vector.reciprocal(zt, zt)
                ot = work.tile([128, 4, O], fp32, tag="ot", bufs=2)
                nc.gpsimd.tensor_tensor(
                    ot, et[:, :, 0:O], zt.broadcast_to([128, 4, O]), ALU.mult
                )
                nc.sync.dma_start(
                    out=out_d[:, 4 * g : 4 * (g + 1), :], in_=ot
                )

            _VAR = _os.environ.get("BODY_VARIANT", "B")

            def body():
                st1 = {}
                st2 = {}
                if _VAR == "A":
                    # S1 one group ahead of S2, one ahead of S3.
                    st1[0] = stage1(0)
                    st1[1] = stage1(1)
                    st2[0] = stage2(0, st1[0])
                    st1[2] = stage1(2)
                    st2[1] = stage2(1, st1[1])
                    stage3(0, st2[0])
                    st1[3] = stage1(3)
                    st2[2] = stage2(2, st1[2])
                    stage3(1, st2[1])
                    st2[3] = stage2(3, st1[3])
                    stage3(2, st2[2])
                    stage3(3, st2[3])
                else:
                    # S1 skewed one ahead; S2+S3 in natural group order.
                    st1[0] = stage1(0)
                    for g in range(NG):
                        if g + 1 < NG:
                            st1[g + 1] = stage1(g + 1)
                        stage3(g, stage2(g, st1[g]))

            if repeat == 1:
                body()
            elif unroll:
                for _ in range(repeat):
                    body()
            else:
                inner = 16 if repeat % 16 == 0 else (4 if repeat % 4 == 0 else 1)
                with tc.For_i(0, repeat // inner, 1, staggered_reset=True):
                    for _ in range(inner):
                        body()

    nc.finalize()
    return nc


def _host_prep(x, mfs, rules):
    x = np.ascontiguousarray(x, dtype=np.float32)
    mfs = np.ascontiguousarray(mfs, dtype=np.float32)
    ra = np.zeros((R1 * R2N, O + 1), dtype=np.float32)
    ra[:, :O] = rules
    ra[:, O] = 1.0
    r2mat = np.zeros((256, W), dtype=np.float32)
    r2mat[:R2N] = ra.reshape(R1, R2N, O + 1).transpose(1, 2, 0).reshape(R2N, W)
    return x, mfs, r2mat


def _make_runner(nc):
    """Jitted 8-core SPMD runner (compiles once, reused across calls)."""
    import jax
    from jax.sharding import Mesh, PartitionSpec
    from jax.experimental.shard_map import shard_map

    import concourse.mybir as mybir
    from concourse import bass2jax
    from concourse.bass2jax import _bass_exec_p, install_neuronx_cc_hook

    install_neuronx_cc_hook()
    partition_name = nc.partition_id_tensor.name if nc.partition_id_tensor else None
    in_names, out_names, out_avals, out_shapes = [], [], [], []
    for alloc in nc.m.functions[0].allocations:
        if not isinstance(alloc, mybir.MemoryLocationSet):
            continue
        name = alloc.memorylocations[0].name
        if alloc.kind == "ExternalInput":
            if name != partition_name:
                in_names.append(name)
        elif alloc.kind == "ExternalOutput":
            out_names.append(name)
            shape = tuple(alloc.tensor_shape)
            dtype = mybir.dt.np(alloc.dtype)
            out_avals.append(jax.core.ShapedArray(shape, dtype))
            out_shapes.append((shape, dtype))
    n_params = len(in_names)
    n_outs = len(out_avals)
    all_in = list(in_names) + list(out_names)
    if partition_name is not None:
        all_in.append(partition_name)
    donate = tuple(range(n_params, n_params + n_outs))

    def _fn(*args):
        operands = list(args)
        if partition_name is not None:
            operands.append(bass2jax.partition_id_tensor())
        return tuple(
            _bass_exec_p.bind(
                *operands,
                out_avals=tuple(out_avals),
                in_names=tuple(all_in),
                out_names=tuple(out_names),
                lowering_input_output_aliases=(),
                sim_require_finite=True,
                sim_require_nnan=True,
                nc=nc,
            )
        )

    devices = jax.devices()[:NC]
    mesh = Mesh(np.asarray(devices), ("core",))
    spec = (PartitionSpec("core"),)
    sharded = jax.jit(
        shard_map(
            _fn, mesh=mesh, in_specs=spec * (n_params + n_outs),
            out_specs=spec * n_outs, check_rep=False,
        ),
        donate_argnums=donate, keep_unused=True,
    )

    def run(in_maps, fetch=True):
        import jax

        concat_in = [
            np.concatenate([np.asarray(m[n]) for m in in_maps], axis=0)
            for n in in_names
        ]
        zeros = [np.zeros((NC * s[0], *s[1:]), dt) for s, dt in out_shapes]
        out_arrs = sharded(*concat_in, *zeros)
        if not fetch:
            jax.block_until_ready(out_arrs)
            return None
        return {
            n: np.asarray(out_arrs[i]).reshape(NC, *out_avals[i].shape)
            for i, n in enumerate(out_names)
        }

    return run


def kernel(x: np.ndarray, mfs: np.ndarray, rules: np.ndarray) -> np.ndarray:
    make_runner = _make_runner
    key = "runner"
    if key not in _CACHE:
        _CACHE[key] = make_runner(_build_program())
    run = _CACHE[key]

    x, mfs, r2mat = _host_prep(x, mfs, rules)
    in_maps = [
        {"xs": x[c * BC : (c + 1) * BC], "mfs": mfs, "r2mat": r2mat}
        for c in range(NC)
    ]
    outs = run(in_maps)
    o = outs["out"]  # [NC, 128, BC//128, O]
    o = o.transpose(0, 2, 1, 3).reshape(B, O)
    return np.ascontiguousarray(o)


# revision 4
# speedup vs baseline: 1.0171x; 1.0164x over previous
"""ExtendedANFIS forward, p=9 factored rules, engine-balanced combine.

    r = r1*243 + r2   (P over inputs {0,1} -> 9 rules, Q over {2..6} -> 243)
    out[b,o] = (sum_r1 P[b,r1] * C[b,(o,r1)]) / (ones-col + eps)
    C[b,(o,r1)] = sum_r2 Q[b,r2] * R2[r2, o*9+r1]

v2 layout/schedule:
- x^T transposes + Lpos Square hoisted to setup (loop-invariant).
- P-matmul PSUM rides in the pq tile's 2nd bank; one Exp covers Q chunk0+P.
- Per-sub combine split: subs 0,2 ACT-evac -> all-bf16 DVE mult (2x);
  subs 1,3 Pool (gpsimd) mult direct from PSUM.
- 9->1 reduction tree all-bf16 on DVE (2x); cb combine on DVE.
"""

import numpy as np

NC = 8
B = 16384
BC = B // NC          # 2048 rows per core
I = 7
M = 3
O = 64
R1 = 9
R2N = 243
KB = 21               # Lpos rows: k = 7m + i (all inputs; P uses i<2)
W = (O + 1) * R1      # 585 real columns, col = o*9 + r1
WA = 56 * R1          # 504 (o 0..55)
WB = W - WA           # 81 (o 56..64 incl ones-col)
NG = 4
GB = 512
EPS = 1e-6

_CACHE = {}


def _build_sel5() -> np.ndarray:
    """[21, 256] f32: row 7m+i (i in 2..6), col r2 -> -1 where digit==m."""
    dg = np.stack(np.unravel_index(np.arange(R2N), (M,) * 5), axis=0)
    sel = np.zeros((KB, 256), dtype=np.float32)
    for m in range(M):
        for j in range(5):
            sel[7 * m + (j + 2), :R2N] = -(dg[j] == m).astype(np.float32)
    return sel


def _build_selp() -> np.ndarray:
    """[21, 10] f32: logP selector; col r1=3*m0+m1 <- -1 at rows (m0,i=0),(m1,i=1)."""
    sel = np.zeros((KB, 10), dtype=np.float32)
    for m0 in range(M):
        for m1 in range(M):
            sel[7 * m0 + 0, 3 * m0 + m1] = -1.0
            sel[7 * m1 + 1, 3 * m0 + m1] = -1.0
    return sel


def _build_program(repeat: int = 1, unroll: bool = False):
    import concourse.mybir as mybir
    import concourse.tile as tile
    from concourse import bacc
    from concourse.bass import DRamTensorHandle

    fp32 = mybir.dt.float32
    bf16 = mybir.dt.bfloat16
    f32r = mybir.dt.float32r
    AF = mybir.ActivationFunctionType
    ALU = mybir.AluOpType

    import os as _os
    _CA_BUFS = int(_os.environ.get("CA_BUFS", "4"))
    _PQ_BUFS = int(_os.environ.get("PQ_BUFS", "2"))
    _T1_SPLIT = _os.environ.get("T1_SPLIT", "0") == "1"

    nc = bacc.Bacc(None, target_bir_lowering=False, debug=False)

    xs = nc.dram_tensor("xs", [BC, I], fp32, kind="ExternalInput")
    mfs_d = nc.dram_tensor("mfs", [I, M, 2], fp32, kind="ExternalInput")
    r2_d = nc.dram_tensor("r2mat", [256, W], fp32, kind="ExternalInput")
    out_d = nc.dram_tensor("out", [128, BC // 128, O], bf16, kind="ExternalOutput")

    sel_np = _build_sel5()
    sel_c = nc.inline_tensor(sel_np, name="sel5")
    nc.lookup_mls(sel_c).dtype = f32r
    sel_c = DRamTensorHandle("sel5", list(sel_np.shape), f32r)
    selp_np = _build_selp()
    selp_c = nc.inline_tensor(selp_np, name="selp")
    nc.lookup_mls(selp_c).dtype = f32r
    selp_c = DRamTensorHandle("selp", list(selp_np.shape), f32r)
    id_c = nc.inline_tensor(np.eye(128, dtype=np.float32), name="ident")

    with tile.TileContext(nc) as tc:
        with (
            tc.tile_pool(name="consts", bufs=1) as consts,
            tc.tile_pool(name="work", bufs=2) as work,
            tc.tile_pool(name="psQ", bufs=2, space="PSUM") as psQ,
            tc.tile_pool(name="psA", bufs=1, space="PSUM") as psA,
            tc.tile_pool(name="psB", bufs=1, space="PSUM") as psB,
        ):
            # ---- constants / input staging (all loop-invariant) ----
            # xallQ[p, g, nt, (m,i)] = xs[(4g+nt)*128+p, 2+i], x replicated
            # 3x (once per m) so one transpose yields all 21 Lpos rows.
            xallQ = consts.tile([128, NG, 4, KB], fp32)
            for g in range(NG):
                for m in range(M):
                    nc.sync.dma_start(
                        out=xallQ[:, g, :, 7 * m : 7 * m + 7],
                        in_=xs[g * GB : (g + 1) * GB, :].rearrange(
                            "(nt p) i -> p nt i", nt=4, p=128
                        ),
                    )
            sel5 = consts.tile([KB, 256], f32r)
            nc.scalar.dma_start(out=sel5[:, 0:128], in_=sel_c[:, 0:128])
            nc.scalar.dma_start(out=sel5[:, 128:], in_=sel_c[:, 128:])
            selp = consts.tile([KB, 10], f32r)
            nc.scalar.dma_start(out=selp, in_=selp_c[:, :])
            ident = consts.tile([128, 128], fp32)
            nc.sync.dma_start(out=ident, in_=id_c[:, :])
            r2f = consts.tile([128, 2, W], fp32)
            for c in range(2):
                nc.gpsimd.dma_start(
                    out=r2f[:, c], in_=r2_d[128 * c : 128 * (c + 1), :]
                )
            r2a = consts.tile([128, 2, WA], bf16)
            nc.vector.tensor_copy(out=r2a, in_=r2f[:, :, 0:WA])
            r2b = consts.tile([128, 2, WB + 1], bf16)
            nc.vector.memset(r2b, 0.0)
            nc.vector.tensor_copy(out=r2b[:, :, 0:WB], in_=r2f[:, :, WA:W])

            # mfs -> c/d per Lpos row (k = 7m + i)
            mtile = consts.tile([KB, 2], fp32)
            for m in range(M):
                nc.scalar.dma_start(
                    out=mtile[7 * m : 7 * m + 7, :], in_=mfs_d[:, m, :]
                )
            tmp = consts.tile([KB, 1], fp32)
            cvec = consts.tile([KB, 1], fp32)
            dvec = consts.tile([KB, 1], fp32)
            nc.vector.tensor_scalar_mul(tmp, mtile[:, 1:2], -1.0)
            nc.vector.tensor_tensor(tmp, mtile[:, 1:2], tmp, ALU.max)
            nc.vector.tensor_scalar_add(tmp, tmp, EPS)
            nc.vector.reciprocal(cvec, tmp)
            nc.vector.tensor_scalar_mul(cvec, cvec, float(1.0 / np.sqrt(2.0)))
            nc.vector.tensor_tensor(dvec, mtile[:, 0:1], cvec, ALU.mult)
            nc.vector.tensor_scalar_mul(dvec, dvec, -1.0)

            # Lpos [21, BC] computed once: x^T via PE transpose, then
            # Square(scale*x + bias) per Lpos row on ACT. px borrows a
            # pq-tagged PSUM buffer (setup only) to save a bank.
            lpos = consts.tile([KB, BC], f32r)
            for g in range(NG):
                pxt = psQ.tile([128, 512], fp32, tag="pq", bufs=_PQ_BUFS)
                px = pxt[0:KB, 0:512].rearrange("p (nt c) -> p nt c", nt=4)
                for nt in range(4):
                    nc.tensor.transpose(px[:, nt], xallQ[:, g, nt, :], ident)
                nc.scalar.activation(
                    out=lpos[:, g * GB : (g + 1) * GB],
                    in_=px[:, :].rearrange("p nt c -> p (nt c)"),
                    func=AF.Square, scale=cvec, bias=dvec,
                )

            def stage1(g):
                """Selector matmuls + exps: logQ chunk0 [128,512] + logP
                [128,4,10] share the pq tile (banks 0/1); one Exp covers
                both. Returns (qp, qt2, ptb)."""
                gs = slice(g * GB, (g + 1) * GB)
                # cbp bank: cb rules-part [0:324] + logP psum [324:364]
                cbp = psB.tile([128, 364], fp32, tag="cbp", bufs=2)
                pq = psQ.tile([128, 512], fp32, tag="pq", bufs=_PQ_BUFS)
                nc.tensor.matmul(
                    pq, sel5[:, 0:128], lpos[:, gs],
                    start=True, stop=True,
                )
                for s in range(4):
                    nc.tensor.matmul(
                        cbp[:, 324 + s * 10 : 324 + s * 10 + 10],
                        lpos[:, g * GB + s * 128 : g * GB + (s + 1) * 128],
                        selp, start=True, stop=True,
                        skip_group_check=True,
                    )
                qp = work.tile([128, 512], bf16, tag="qp", bufs=3)
                nc.scalar.activation(out=qp, in_=pq, func=AF.Exp)
                ptb = work.tile([128, 4, 10], bf16, tag="ptb", bufs=3)
                nc.scalar.activation(
                    out=ptb, in_=cbp[:, 324:364].rearrange(
                        "p (s r) -> p s r", s=4), func=AF.Exp)
                pq2 = psQ.tile([128, 512], fp32, tag="pq", bufs=_PQ_BUFS)
                nc.tensor.matmul(
                    pq2, sel5[:, 128:256], lpos[:, gs],
                    start=True, stop=True,
                )
                qt2 = work.tile([128, 512], bf16, tag="qt2", bufs=3)
                nc.scalar.activation(out=qt2, in_=pq2, func=AF.Exp)
                return qp, qt2, ptb, cbp

            def stage2(g, st):
                """C matmuls + per-sub combine: s0,s2 Pool direct-from-PSUM;
                s1,s3 ACT evac + DVE bf16 mult (2x); cb Pool-evac + DVE
                mult. Returns (dtile, ptb)."""
                qp, qt2, ptb, cbp = st
                dtile = work.tile([128, 4, W], bf16, tag="d", bufs=3)
                cb = cbp[:, 0:324].rearrange("p (s w) -> p s w", s=4)
                for s in range(4):
                    ca = psA.tile([128, 512], fp32, tag="ca", bufs=_CA_BUFS)
                    for c in range(2):
                        lhsT = (qp, qt2)[c][:, s * 128 : (s + 1) * 128]
                        nc.tensor.matmul(
                            ca[:, 0:WA], lhsT, r2a[:, c],
                            start=(c == 0), stop=(c == 1),
                            skip_group_check=True,
                        )
                        nc.tensor.matmul(
                            cb[:, s], lhsT, r2b[:, c, 0:WB],
                            start=(c == 0), stop=(c == 1),
                            skip_group_check=True,
                        )
                    pba = (
                        ptb[:, s, 0:R1].unsqueeze(1)
                        .broadcast_to([128, 56, R1])
                    )
                    if s in (0, 1):
                        # DVE mult direct from PSUM (Pool cannot touch PSUM)
                        nc.vector.tensor_tensor(
                            dtile[:, s, 0:WA].rearrange(
                                "p (o r) -> p o r", r=R1
                            ),
                            ca[:, 0:WA].rearrange("p (o r) -> p o r", r=R1),
                            pba, ALU.mult,
                        )
                    else:
                        # ACT evacuates psum->bf16; Pool bf16 mult (SBUF-only)
                        cev = work.tile([128, WA], bf16, tag="cev", bufs=3)
                        nc.scalar.copy(out=cev, in_=ca[:, 0:WA])
                        nc.gpsimd.tensor_tensor(
                            dtile[:, s, 0:WA].rearrange(
                                "p (o r) -> p o r", r=R1
                            ),
                            cev[:, :].rearrange("p (o r) -> p o r", r=R1),
                            pba, ALU.mult,
                        )
                if _T1_SPLIT:
                    t1 = work.tile([128, 4, O + 1, 4], bf16, tag="t1", bufs=2)
                    for s in range(4):
                        dvs = dtile[:, s, :].rearrange(
                            "p (o r) -> p o r", r=R1)
                        nc.vector.tensor_tensor(
                            t1[:, s], dvs[:, :, 0:4], dvs[:, :, 4:8], ALU.add
                        )
                else:
                    t1 = None
                # cb: ACT evac -> DVE bf16 mult (2x)
                cbe = work.tile([128, 4, WB], bf16, tag="cbe", bufs=2)
                nc.scalar.copy(out=cbe, in_=cb)
                nc.vector.tensor_tensor(
                    dtile[:, :, WA:W].rearrange("p s (o r) -> p s o r", r=R1),
                    cbe.rearrange("p s (o r) -> p s o r", r=R1),
                    ptb[:, :, 0:R1].unsqueeze(2).broadcast_to(
                        [128, 4, 9, R1]
                    ),
                    ALU.mult,
                )
                return dtile, t1

            def stage3(g, st2):
                """9->1 tree all-bf16 (DVE 2x) + normalize + store."""
                dtile, t1 = st2
                dv = dtile[:, :, :].rearrange("p s (o r) -> p s o r", r=R1)
                t2 = work.tile([128, 4, O + 1, 2], bf16, tag="t2", bufs=2)
                e1 = work.tile([128, 4, O + 1], bf16, tag="e1", bufs=2)
                et = work.tile([128, 4, O + 1], bf16, tag="et", bufs=2)
                if t1 is None:
                    t1 = work.tile([128, 4, O + 1, 4], bf16, tag="t1", bufs=2)
                    nc.vector.tensor_tensor(
                        t1, dv[:, :, :, 0:4], dv[:, :, :, 4:8], ALU.add
                    )
                nc.vector.tensor_tensor(
                    t2, t1[:, :, :, 0:2], t1[:, :, :, 2:4], ALU.add
                )
                nc.vector.tensor_tensor(
                    e1, t2[:, :, :, 0], t2[:, :, :, 1], ALU.add
                )
                nc.vector.tensor_tensor(et, e1, dv[:, :, :, 8], ALU.add)
                zt = work.tile([128, 4, 1], fp32, tag="zt", bufs=2)
                nc.vector.tensor_scalar_add(zt, et[:, :, 64:65], EPS)
                nc.vector.reciprocal(zt, zt)
                ot = work.tile([128, 4, O], bf16, tag="ot", bufs=2)
                nc.gpsimd.tensor_tensor(
                    ot, et[:, :, 0:O], zt.broadcast_to([128, 4, O]), ALU.mult
                )
                nc.sync.dma_start(
                    out=out_d[:, 4 * g : 4 * (g + 1), :].rearrange(
                        "p j o -> p (j o)"),
                    in_=ot.rearrange("p j o -> p (j o)"),
                )

            _VAR = _os.environ.get("BODY_VARIANT", "B")

            def body():
                st1 = {}
                st2 = {}
                if _VAR == "A":
                    # S1 one group ahead of S2, one ahead of S3.
                    st1[0] = stage1(0)
                    st1[1] = stage1(1)
                    st2[0] = stage2(0, st1[0])
                    st1[2] = stage1(2)
                    st2[1] = stage2(1, st1[1])
                    stage3(0, st2[0])
                    st1[3] = stage1(3)
                    st2[2] = stage2(2, st1[2])
                    stage3(1, st2[1])
                    st2[3] = stage2(3, st1[3])
                    stage3(2, st2[2])
                    stage3(3, st2[3])
                else:
                    # S1 skewed one ahead; S2+S3 in natural group order.
                    st1[0] = stage1(0)
                    for g in range(NG):
                        if g + 1 < NG:
                            st1[g + 1] = stage1(g + 1)
                        stage3(g, stage2(g, st1[g]))

            if repeat == 1:
                body()
            elif unroll:
                for _ in range(repeat):
                    body()
            else:
                inner = 16 if repeat % 16 == 0 else (4 if repeat % 4 == 0 else 1)
                with tc.For_i(0, repeat // inner, 1, staggered_reset=True):
                    for _ in range(inner):
                        body()

    nc.finalize()
    return nc


def _host_prep(x, mfs, rules):
    x = np.ascontiguousarray(x, dtype=np.float32)
    mfs = np.ascontiguousarray(mfs, dtype=np.float32)
    ra = np.zeros((R1 * R2N, O + 1), dtype=np.float32)
    ra[:, :O] = rules
    ra[:, O] = 1.0
    r2mat = np.zeros((256, W), dtype=np.float32)
    r2mat[:R2N] = ra.reshape(R1, R2N, O + 1).transpose(1, 2, 0).reshape(R2N, W)
    return x, mfs, r2mat


def _make_runner(nc):
    """Jitted 8-core SPMD runner (compiles once, reused across calls)."""
    import jax
    from jax.sharding import Mesh, PartitionSpec
    from jax.experimental.shard_map import shard_map

    import concourse.mybir as mybir
    from concourse import bass2jax
    from concourse.bass2jax import _bass_exec_p, install_neuronx_cc_hook

    install_neuronx_cc_hook()
    partition_name = nc.partition_id_tensor.name if nc.partition_id_tensor else None
    in_names, out_names, out_avals, out_shapes = [], [], [], []
    for alloc in nc.m.functions[0].allocations:
        if not isinstance(alloc, mybir.MemoryLocationSet):
            continue
        name = alloc.memorylocations[0].name
        if alloc.kind == "ExternalInput":
            if name != partition_name:
                in_names.append(name)
        elif alloc.kind == "ExternalOutput":
            out_names.append(name)
            shape = tuple(alloc.tensor_shape)
            dtype = mybir.dt.np(alloc.dtype)
            out_avals.append(jax.core.ShapedArray(shape, dtype))
            out_shapes.append((shape, dtype))
    n_params = len(in_names)
    n_outs = len(out_avals)
    all_in = list(in_names) + list(out_names)
    if partition_name is not None:
        all_in.append(partition_name)
    donate = tuple(range(n_params, n_params + n_outs))

    def _fn(*args):
        operands = list(args)
        if partition_name is not None:
            operands.append(bass2jax.partition_id_tensor())
        return tuple(
            _bass_exec_p.bind(
                *operands,
                out_avals=tuple(out_avals),
                in_names=tuple(all_in),
                out_names=tuple(out_names),
                lowering_input_output_aliases=(),
                sim_require_finite=True,
                sim_require_nnan=True,
                nc=nc,
            )
        )

    devices = jax.devices()[:NC]
    mesh = Mesh(np.asarray(devices), ("core",))
    spec = (PartitionSpec("core"),)
    sharded = jax.jit(
        shard_map(
            _fn, mesh=mesh, in_specs=spec * (n_params + n_outs),
            out_specs=spec * n_outs, check_rep=False,
        ),
        donate_argnums=donate, keep_unused=True,
    )

    def run(in_maps, fetch=True):
        import jax

        concat_in = [
            np.concatenate([np.asarray(m[n]) for m in in_maps], axis=0)
            for n in in_names
        ]
        zeros = [np.zeros((NC * s[0], *s[1:]), dt) for s, dt in out_shapes]
        out_arrs = sharded(*concat_in, *zeros)
        if not fetch:
            jax.block_until_ready(out_arrs)
            return None
        return {
            n: np.asarray(out_arrs[i]).reshape(NC, *out_avals[i].shape)
            for i, n in enumerate(out_names)
        }

    return run


def kernel(x: np.ndarray, mfs: np.ndarray, rules: np.ndarray) -> np.ndarray:
    make_runner = _make_runner
    key = "runner"
    if key not in _CACHE:
        _CACHE[key] = make_runner(_build_program())
    run = _CACHE[key]

    x, mfs, r2mat = _host_prep(x, mfs, rules)
    in_maps = [
        {"xs": x[c * BC : (c + 1) * BC], "mfs": mfs, "r2mat": r2mat}
        for c in range(NC)
    ]
    outs = run(in_maps)
    o = outs["out"]  # [NC, 128, BC//128, O] bf16
    o = o.astype(np.float32).transpose(0, 2, 1, 3).reshape(B, O)
    return np.ascontiguousarray(o)


# revision 6
# speedup vs baseline: 1.1914x; 1.1714x over previous
"""ExtendedANFIS forward, p=9 factored rules, engine-balanced combine.

    r = r1*243 + r2   (P over inputs {0,1} -> 9 rules, Q over {2..6} -> 243)
    out[b,o] = (sum_r1 P[b,r1] * C[b,(o,r1)]) / (ones-col + eps)
    C[b,(o,r1)] = sum_r2 Q[b,r2] * R2[r2, o*9+r1]

v2 layout/schedule:
- x^T transposes + Lpos Square hoisted to setup (loop-invariant).
- P-matmul PSUM rides in the pq tile's 2nd bank; one Exp covers Q chunk0+P.
- Per-sub combine split: subs 0,2 ACT-evac -> all-bf16 DVE mult (2x);
  subs 1,3 Pool (gpsimd) mult direct from PSUM.
- 9->1 reduction tree all-bf16 on DVE (2x); cb combine on DVE.
"""

import numpy as np

NC = 8
B = 16384
BC = B // NC          # 2048 rows per core
I = 7
M = 3
O = 64
R1 = 9
R2N = 243
KB = 21               # Lpos rows: k = 7m + i (all inputs; P uses i<2)
W = (O + 1) * R1      # 585 real columns, col = o*9 + r1
WA = 56 * R1          # 504 (o 0..55)
WB = W - WA           # 81 (o 56..64 incl ones-col)
NG = 4
GB = 512
EPS = 1e-6

_CACHE = {}


def _build_sel5() -> np.ndarray:
    """[21, 256] f32: row 7m+i (i in 2..6), col r2 -> -1 where digit==m."""
    dg = np.stack(np.unravel_index(np.arange(R2N), (M,) * 5), axis=0)
    sel = np.zeros((KB, 256), dtype=np.float32)
    for m in range(M):
        for j in range(5):
            sel[7 * m + (j + 2), :R2N] = -(dg[j] == m).astype(np.float32)
    return sel


def _build_selp() -> np.ndarray:
    """[21, 10] f32: logP selector; col r1=3*m0+m1 <- -1 at rows (m0,i=0),(m1,i=1)."""
    sel = np.zeros((KB, 10), dtype=np.float32)
    for m0 in range(M):
        for m1 in range(M):
            sel[7 * m0 + 0, 3 * m0 + m1] = -1.0
            sel[7 * m1 + 1, 3 * m0 + m1] = -1.0
    return sel


def _build_program(repeat: int = 1, unroll: bool = False):
    import concourse.mybir as mybir
    import concourse.tile as tile
    from concourse import bacc
    from concourse.bass import DRamTensorHandle

    fp32 = mybir.dt.float32
    bf16 = mybir.dt.bfloat16
    f32r = mybir.dt.float32r
    AF = mybir.ActivationFunctionType
    ALU = mybir.AluOpType

    import os as _os
    _CA_BUFS = int(_os.environ.get("CA_BUFS", "4"))
    _PQ_BUFS = int(_os.environ.get("PQ_BUFS", "2"))
    _T1_SPLIT = _os.environ.get("T1_SPLIT", "0") == "1"
    _POOL_MULT = _os.environ.get("POOL_MULT", "1") == "1"
    _POOL_OT = _os.environ.get("POOL_OT", "1") == "1"

    nc = bacc.Bacc(None, target_bir_lowering=False, debug=False)

    xs = nc.dram_tensor("xs", [BC, I], fp32, kind="ExternalInput")
    mfs_d = nc.dram_tensor("mfs", [I, M, 2], fp32, kind="ExternalInput")
    r2_d = nc.dram_tensor("r2mat", [256, W], fp32, kind="ExternalInput")
    out_d = nc.dram_tensor("out", [128, BC // 128, O], bf16, kind="ExternalOutput")

    sel_np = _build_sel5()
    sel_c = nc.inline_tensor(sel_np, name="sel5")
    nc.lookup_mls(sel_c).dtype = f32r
    sel_c = DRamTensorHandle("sel5", list(sel_np.shape), f32r)
    selp_np = _build_selp()
    selp_c = nc.inline_tensor(selp_np, name="selp")
    nc.lookup_mls(selp_c).dtype = f32r
    selp_c = DRamTensorHandle("selp", list(selp_np.shape), f32r)
    id_c = nc.inline_tensor(np.eye(128, dtype=np.float32), name="ident")

    with tile.TileContext(nc) as tc:
        with (
            tc.tile_pool(name="consts", bufs=1) as consts,
            tc.tile_pool(name="work", bufs=2) as work,
            tc.tile_pool(name="psQ", bufs=2, space="PSUM") as psQ,
            tc.tile_pool(name="psA", bufs=1, space="PSUM") as psA,
            tc.tile_pool(name="psB", bufs=1, space="PSUM") as psB,
        ):
            # ---- constants / input staging (all loop-invariant) ----
            # xallQ[p, g, nt, (m,i)] = xs[(4g+nt)*128+p, 2+i], x replicated
            # 3x (once per m) so one transpose yields all 21 Lpos rows.
            xallQ = consts.tile([128, NG, 4, KB], fp32)
            for g in range(NG):
                for m in range(M):
                    nc.sync.dma_start(
                        out=xallQ[:, g, :, 7 * m : 7 * m + 7],
                        in_=xs[g * GB : (g + 1) * GB, :].rearrange(
                            "(nt p) i -> p nt i", nt=4, p=128
                        ),
                    )
            sel5 = consts.tile([KB, 256], f32r)
            nc.scalar.dma_start(out=sel5[:, 0:128], in_=sel_c[:, 0:128])
            nc.scalar.dma_start(out=sel5[:, 128:], in_=sel_c[:, 128:])
            selp = consts.tile([KB, 10], f32r)
            nc.scalar.dma_start(out=selp, in_=selp_c[:, :])
            ident = consts.tile([128, 128], fp32)
            nc.sync.dma_start(out=ident, in_=id_c[:, :])
            r2f = consts.tile([128, 2, W], fp32)
            for c in range(2):
                nc.gpsimd.dma_start(
                    out=r2f[:, c], in_=r2_d[128 * c : 128 * (c + 1), :]
                )
            r2a = consts.tile([128, 2, WA], bf16)
            nc.vector.tensor_copy(out=r2a, in_=r2f[:, :, 0:WA])
            r2b = consts.tile([128, 2, WB + 1], bf16)
            nc.vector.memset(r2b, 0.0)
            nc.vector.tensor_copy(out=r2b[:, :, 0:WB], in_=r2f[:, :, WA:W])

            # mfs -> c/d per Lpos row (k = 7m + i)
            mtile = consts.tile([KB, 2], fp32)
            for m in range(M):
                nc.scalar.dma_start(
                    out=mtile[7 * m : 7 * m + 7, :], in_=mfs_d[:, m, :]
                )
            tmp = consts.tile([KB, 1], fp32)
            cvec = consts.tile([KB, 1], fp32)
            dvec = consts.tile([KB, 1], fp32)
            nc.vector.tensor_scalar_mul(tmp, mtile[:, 1:2], -1.0)
            nc.vector.tensor_tensor(tmp, mtile[:, 1:2], tmp, ALU.max)
            nc.vector.tensor_scalar_add(tmp, tmp, EPS)
            nc.vector.reciprocal(cvec, tmp)
            nc.vector.tensor_scalar_mul(cvec, cvec, float(1.0 / np.sqrt(2.0)))
            nc.vector.tensor_tensor(dvec, mtile[:, 0:1], cvec, ALU.mult)
            nc.vector.tensor_scalar_mul(dvec, dvec, -1.0)

            # Lpos [21, BC] computed once: x^T via PE transpose, then
            # Square(scale*x + bias) per Lpos row on ACT. px borrows a
            # pq-tagged PSUM buffer (setup only) to save a bank.
            lpos = consts.tile([KB, BC], f32r)
            for g in range(NG):
                pxt = psQ.tile([128, 512], fp32, tag="pq", bufs=_PQ_BUFS)
                px = pxt[0:KB, 0:512].rearrange("p (nt c) -> p nt c", nt=4)
                for nt in range(4):
                    nc.tensor.transpose(px[:, nt], xallQ[:, g, nt, :], ident)
                nc.scalar.activation(
                    out=lpos[:, g * GB : (g + 1) * GB],
                    in_=px[:, :].rearrange("p nt c -> p (nt c)"),
                    func=AF.Square, scale=cvec, bias=dvec,
                )

            def stage1(g):
                """Selector matmuls + exps: logQ chunk0 [128,512] + logP
                [128,4,10] share the pq tile (banks 0/1); one Exp covers
                both. Returns (qp, qt2, ptb)."""
                gs = slice(g * GB, (g + 1) * GB)
                # cbp bank: cb rules-part [0:324] + logP psum [324:364]
                cbp = psB.tile([128, 364], fp32, tag="cbp", bufs=2)
                pq = psQ.tile([128, 512], fp32, tag="pq", bufs=_PQ_BUFS)
                nc.tensor.matmul(
                    pq, sel5[:, 0:128], lpos[:, gs],
                    start=True, stop=True,
                )
                for s in range(4):
                    nc.tensor.matmul(
                        cbp[:, 324 + s * 10 : 324 + s * 10 + 10],
                        lpos[:, g * GB + s * 128 : g * GB + (s + 1) * 128],
                        selp, start=True, stop=True,
                        skip_group_check=True,
                    )
                qp = work.tile([128, 512], bf16, tag="qp", bufs=3)
                nc.scalar.activation(out=qp, in_=pq, func=AF.Exp)
                ptb = work.tile([128, 4, 10], bf16, tag="ptb", bufs=3)
                nc.scalar.activation(
                    out=ptb, in_=cbp[:, 324:364].rearrange(
                        "p (s r) -> p s r", s=4), func=AF.Exp)
                pq2 = psQ.tile([128, 512], fp32, tag="pq", bufs=_PQ_BUFS)
                nc.tensor.matmul(
                    pq2, sel5[:, 128:256], lpos[:, gs],
                    start=True, stop=True,
                )
                qt2 = work.tile([128, 512], bf16, tag="qt2", bufs=3)
                nc.scalar.activation(out=qt2, in_=pq2, func=AF.Exp)
                return qp, qt2, ptb, cbp

            def stage2(g, st):
                """C matmuls + per-sub combine: s0,s2 Pool direct-from-PSUM;
                s1,s3 ACT evac + DVE bf16 mult (2x); cb Pool-evac + DVE
                mult. Returns (dtile, ptb)."""
                qp, qt2, ptb, cbp = st
                dtile = work.tile([128, 4, W], bf16, tag="d", bufs=3)
                cb = cbp[:, 0:324].rearrange("p (s w) -> p s w", s=4)
                for sp in range(2):
                    # sub-pair (2sp, 2sp+1): one 2-bank PSUM tile, one
                    # combine op for both subs.
                    ca2 = psA.tile([128, 2, 512], fp32, tag="ca", bufs=2)
                    for si in range(2):
                        s = 2 * sp + si
                        for c in range(2):
                            lhsT = (qp, qt2)[c][:, s * 128 : (s + 1) * 128]
                            nc.tensor.matmul(
                                ca2[:, si, 0:WA], lhsT, r2a[:, c],
                                start=(c == 0), stop=(c == 1),
                                skip_group_check=True,
                            )
                            nc.tensor.matmul(
                                cb[:, s], lhsT, r2b[:, c, 0:WB],
                                start=(c == 0), stop=(c == 1),
                                skip_group_check=True,
                            )
                    ss = slice(2 * sp, 2 * sp + 2)
                    pba2 = (
                        ptb[:, ss, 0:R1].unsqueeze(2)
                        .broadcast_to([128, 2, 56, R1])
                    )
                    if sp == 1:
                        # DVE mult direct from PSUM (Pool cannot touch PSUM)
                        nc.vector.tensor_tensor(
                            dtile[:, ss, 0:WA].rearrange(
                                "p s (o r) -> p s o r", r=R1
                            ),
                            ca2[:, :, 0:WA].rearrange(
                                "p s (o r) -> p s o r", r=R1
                            ),
                            pba2, ALU.mult,
                        )
                    else:
                        # ACT evacuates psum->bf16; Pool bf16 mult (SBUF-only)
                        cev = work.tile([128, 2, WA], bf16, tag="cev", bufs=2)
                        nc.scalar.copy(out=cev, in_=ca2[:, :, 0:WA])
                        eng = nc.gpsimd if _POOL_MULT else nc.vector
                        eng.tensor_tensor(
                            dtile[:, ss, 0:WA].rearrange(
                                "p s (o r) -> p s o r", r=R1
                            ),
                            cev.rearrange("p s (o r) -> p s o r", r=R1),
                            pba2, ALU.mult,
                        )
                if _T1_SPLIT:
                    t1 = work.tile([128, 4, O + 1, 4], bf16, tag="t1", bufs=2)
                    for sp in range(2):
                        dvs = dtile[:, 2 * sp : 2 * sp + 2, :].rearrange(
                            "p s (o r) -> p s o r", r=R1)
                        nc.vector.tensor_tensor(
                            t1[:, 2 * sp : 2 * sp + 2],
                            dvs[:, :, :, 0:4], dvs[:, :, :, 4:8], ALU.add
                        )
                else:
                    t1 = None
                # cb: ACT evac -> DVE bf16 mult (2x)
                cbe = work.tile([128, 4, WB], bf16, tag="cbe", bufs=2)
                nc.scalar.copy(out=cbe, in_=cb)
                nc.vector.tensor_tensor(
                    dtile[:, :, WA:W].rearrange("p s (o r) -> p s o r", r=R1),
                    cbe.rearrange("p s (o r) -> p s o r", r=R1),
                    ptb[:, :, 0:R1].unsqueeze(2).broadcast_to(
                        [128, 4, 9, R1]
                    ),
                    ALU.mult,
                )
                return dtile, t1

            def stage3(g, st2):
                """9->1 tree all-bf16 (DVE 2x) + normalize + store."""
                dtile, t1 = st2
                dv = dtile[:, :, :].rearrange("p s (o r) -> p s o r", r=R1)
                t2 = work.tile([128, 4, O + 1, 2], bf16, tag="t2", bufs=2)
                e1 = work.tile([128, 4, O + 1], bf16, tag="e1", bufs=2)
                et = work.tile([128, 4, O + 1], bf16, tag="et", bufs=2)
                if t1 is None:
                    t1 = work.tile([128, 4, O + 1, 4], bf16, tag="t1", bufs=2)
                    nc.vector.tensor_tensor(
                        t1, dv[:, :, :, 0:4], dv[:, :, :, 4:8], ALU.add
                    )
                nc.vector.tensor_tensor(
                    t2, t1[:, :, :, 0:2], t1[:, :, :, 2:4], ALU.add
                )
                nc.vector.tensor_tensor(
                    e1, t2[:, :, :, 0], t2[:, :, :, 1], ALU.add
                )
                nc.vector.tensor_tensor(et, e1, dv[:, :, :, 8], ALU.add)
                zt = work.tile([128, 4, 1], fp32, tag="zt", bufs=2)
                nc.vector.tensor_scalar_add(zt, et[:, :, 64:65], EPS)
                nc.vector.reciprocal(zt, zt)
                ot = work.tile([128, 4, O], bf16, tag="ot", bufs=2)
                (nc.gpsimd if _POOL_OT else nc.vector).tensor_tensor(
                    ot, et[:, :, 0:O], zt.broadcast_to([128, 4, O]), ALU.mult
                )
                nc.sync.dma_start(
                    out=out_d[:, 4 * g : 4 * (g + 1), :].rearrange(
                        "p j o -> p (j o)"),
                    in_=ot.rearrange("p j o -> p (j o)"),
                )

            _VAR = _os.environ.get("BODY_VARIANT", "B")

            def body():
                st1 = {}
                st2 = {}
                if _VAR == "A":
                    # S1 one group ahead of S2, one ahead of S3.
                    st1[0] = stage1(0)
                    st1[1] = stage1(1)
                    st2[0] = stage2(0, st1[0])
                    st1[2] = stage1(2)
                    st2[1] = stage2(1, st1[1])
                    stage3(0, st2[0])
                    st1[3] = stage1(3)
                    st2[2] = stage2(2, st1[2])
                    stage3(1, st2[1])
                    st2[3] = stage2(3, st1[3])
                    stage3(2, st2[2])
                    stage3(3, st2[3])
                else:
                    # S1 skewed one ahead; S2+S3 in natural group order.
                    st1[0] = stage1(0)
                    for g in range(NG):
                        if g + 1 < NG:
                            st1[g + 1] = stage1(g + 1)
                        stage3(g, stage2(g, st1[g]))

            if repeat == 1:
                body()
            elif unroll:
                for _ in range(repeat):
                    body()
            else:
                inner = int(_os.environ.get("INNER", "0")) or (
                    64 if repeat % 64 == 0 else
                    (16 if repeat % 16 == 0 else (4 if repeat % 4 == 0 else 1)))
                with tc.For_i(0, repeat // inner, 1, staggered_reset=True):
                    for _ in range(inner):
                        body()

    nc.finalize()
    return nc


def _host_prep(x, mfs, rules):
    x = np.ascontiguousarray(x, dtype=np.float32)
    mfs = np.ascontiguousarray(mfs, dtype=np.float32)
    ra = np.zeros((R1 * R2N, O + 1), dtype=np.float32)
    ra[:, :O] = rules
    ra[:, O] = 1.0
    r2mat = np.zeros((256, W), dtype=np.float32)
    r2mat[:R2N] = ra.reshape(R1, R2N, O + 1).transpose(1, 2, 0).reshape(R2N, W)
    return x, mfs, r2mat


def _make_runner(nc):
    """Jitted 8-core SPMD runner (compiles once, reused across calls)."""
    import jax
    from jax.sharding import Mesh, PartitionSpec
    from jax.experimental.shard_map import shard_map

    import concourse.mybir as mybir
    from concourse import bass2jax
    from concourse.bass2jax import _bass_exec_p, install_neuronx_cc_hook

    install_neuronx_cc_hook()
    partition_name = nc.partition_id_tensor.name if nc.partition_id_tensor else None
    in_names, out_names, out_avals, out_shapes = [], [], [], []
    for alloc in nc.m.functions[0].allocations:
        if not isinstance(alloc, mybir.MemoryLocationSet):
            continue
        name = alloc.memorylocations[0].name
        if alloc.kind == "ExternalInput":
            if name != partition_name:
                in_names.append(name)
        elif alloc.kind == "ExternalOutput":
            out_names.append(name)
            shape = tuple(alloc.tensor_shape)
            dtype = mybir.dt.np(alloc.dtype)
            out_avals.append(jax.core.ShapedArray(shape, dtype))
            out_shapes.append((shape, dtype))
    n_params = len(in_names)
    n_outs = len(out_avals)
    all_in = list(in_names) + list(out_names)
    if partition_name is not None:
        all_in.append(partition_name)
    donate = tuple(range(n_params, n_params + n_outs))

    def _fn(*args):
        operands = list(args)
        if partition_name is not None:
            operands.append(bass2jax.partition_id_tensor())
        return tuple(
            _bass_exec_p.bind(
                *operands,
                out_avals=tuple(out_avals),
                in_names=tuple(all_in),
                out_names=tuple(out_names),
                lowering_input_output_aliases=(),
                sim_require_finite=True,
                sim_require_nnan=True,
                nc=nc,
            )
        )

    devices = jax.devices()[:NC]
    mesh = Mesh(np.asarray(devices), ("core",))
    spec = (PartitionSpec("core"),)
    sharded = jax.jit(
        shard_map(
            _fn, mesh=mesh, in_specs=spec * (n_params + n_outs),
            out_specs=spec * n_outs, check_rep=False,
        ),
        donate_argnums=donate, keep_unused=True,
    )

    def run(in_maps, fetch=True):
        import jax

        concat_in = [
            np.concatenate([np.asarray(m[n]) for m in in_maps], axis=0)
            for n in in_names
        ]
        zeros = [np.zeros((NC * s[0], *s[1:]), dt) for s, dt in out_shapes]
        out_arrs = sharded(*concat_in, *zeros)
        if not fetch:
            jax.block_until_ready(out_arrs)
            return None
        return {
            n: np.asarray(out_arrs[i]).reshape(NC, *out_avals[i].shape)
            for i, n in enumerate(out_names)
        }

    return run


def kernel(x: np.ndarray, mfs: np.ndarray, rules: np.ndarray) -> np.ndarray:
    make_runner = _make_runner
    key = "runner"
    if key not in _CACHE:
        _CACHE[key] = make_runner(_build_program())
    run = _CACHE[key]

    x, mfs, r2mat = _host_prep(x, mfs, rules)
    in_maps = [
        {"xs": x[c * BC : (c + 1) * BC], "mfs": mfs, "r2mat": r2mat}
        for c in range(NC)
    ]
    outs = run(in_maps)
    o = outs["out"]  # [NC, 128, BC//128, O] bf16
    o = o.astype(np.float32).transpose(0, 2, 1, 3).reshape(B, O)
    return np.ascontiguousarray(o)
